# revision 32
# baseline (speedup 1.0000x reference)
"""Trainium2 Bass kernel for nn_CNF_76355928588411.

Data-parallel over N across 8 NeuronCores. The tiny t-conditioned hypernet
(three dense layers -> W, U, gate, B; ~6.6 MFLOP, depends only on the scalar
t) is evaluated once on the host in fp32 and its ~49KB output is replicated
to all cores per the sharding hint. The N-compute (h = tanh(x@W^T + B)
[E=64, N], dx = h^T@U/E, plus the Jacobian-trace column; ~4.3 GFLOP and
~130MB of I/O) runs on the devices.

Per-core device pipeline, per window of 1024 samples (2 subs x 512 cols):
  mm1 x2 (f32r, K=128 zero-padded weight halves) -> psum hp [64, 1024]
  ACT tanh(+B per-partition bias)                -> H[0:64]    (fp16)
  DVE square (cross-partition-offset write)      -> H[64:128] = h^2 (fp16)
  mm2 x8: lhsT = H[:, 128c:128c+128] (fp16, K=128, M=128 samples),
          rhs = up' [128, 65] fp16               -> psum [128sample, 65ch]
  DVE tensor-tensor add (+const tile carrying -mean(wu) in channel 64)
                                                 -> out sbuf [128, 520] f32
  DMA out (2080B contiguous per partition via 8-way sample interleave)

up' rows 0:64 = [U/E | 0], rows 64:128 = [0 | wu/E], so one K=128 matmul
emits dx and the h^2-weighted Jacobian column together. All matmuls are
plain 128x128 mode (no tile_position) so the PE never switches tiling
modes. x stays f32r (fp16 x halves DMA but measured 1.38e-3 rel err vs
9.2e-4; kept the safer dtype). Host pre-permutes x so device loads are
plain slices and output stores are contiguous runs; x-batch DMAs are
issued ahead of the constant loads and a dummy tanh hoists the ACT table
load to t=0. TimelineSim: ~60.3us/core; DMA-bound (48us busy).
"""

import sys

sys.path.insert(0, "/opt/trn_rl_repo")

import numpy as np

import concourse.bass as bass
from concourse import bacc
import concourse.mybir as mybir
import concourse.tile as tile
from concourse.bass_utils import run_bass_kernel_spmd

F32 = mybir.dt.float32
F32R = mybir.dt.float32r
F16 = mybir.dt.float16
AF = mybir.ActivationFunctionType

E, D, H_DIM, N = 64, 64, 512, 262144
BLOCK = E * D
OUT_DIM = 3 * BLOCK + E
NCORES = 8
NSH = N // NCORES          # 32768 samples per core
WIN = 1024                 # samples per window
NWIN = NSH // WIN          # 32 windows
WQ = 2                     # windows per DMA batch
NQ = NWIN // WQ            # 8 DMA batches
CH = D + 1                 # 65 output channels

_CACHED = {}


def _build_nc():
    nc = bacc.Bacc("TRN2", target_bir_lowering=False, debug=False,
                   num_devices=NCORES)
    xt = nc.dram_tensor("xt", [128, NSH // 2], F32R, kind="ExternalInput")
    wtd = nc.dram_tensor("wtd", [128, 2 * D], F32R, kind="ExternalInput")
    up = nc.dram_tensor("up", [128, CH], F16, kind="ExternalInput")
    bvec = nc.dram_tensor("bvec", [E, 1], F32, kind="ExternalInput")
    cb = nc.dram_tensor("cb", [128, 8 * CH], F32, kind="ExternalInput")
    out = nc.dram_tensor("out", [NSH, CH], F32, kind="ExternalOutput")

    # out row = 1024*(q*WQ+i) + 8*p + c
    out_r = out.ap().rearrange("(q i p c) ch -> q p i c ch", i=WQ, p=128, c=8)

    with tile.TileContext(nc) as tc:
        with (
            tc.tile_pool(name="consts", bufs=1) as consts,
            tc.tile_pool(name="xin", bufs=10) as xin,
            tc.tile_pool(name="hh", bufs=3) as hhp,
            tc.tile_pool(name="outp", bufs=6) as outp,
            tc.tile_pool(name="ps_h", bufs=2, space="PSUM") as ps_h,
            tc.tile_pool(name="ps_o", bufs=2, space="PSUM") as ps_o,
        ):
            wtd_t = consts.tile([128, 2 * D], F32R)  # cols 0:64=[WT;0], 64:128=[0;WT]
            up_t = consts.tile([128, CH], F16)
            bvec_t = consts.tile([E, 1], F32)
            cb_t = consts.tile([128, 8 * CH], F32)
            xqs = {}

            def fetch(q):
                xq_t = xin.tile([128, WQ * 512], F32R, tag="xq")
                xqs[q] = xq_t
                nc.sync.dma_start(
                    out=xq_t, in_=xt[:, q * WQ * 512:(q + 1) * WQ * 512]
                )

            fetch(0)
            fetch(1)
            dummy = consts.tile([1, 2], F32)
            nc.vector.memset(dummy, 0.0)
            nc.scalar.activation(dummy[:, 1:2], dummy[:, 0:1], AF.Tanh)
            nc.sync.dma_start(out=wtd_t, in_=wtd[:, :])
            nc.sync.dma_start(out=up_t, in_=up[:, :])
            nc.sync.dma_start(out=bvec_t, in_=bvec[:, :])
            nc.sync.dma_start(out=cb_t, in_=cb[:, :])

            for q in range(NQ):
                if q + 2 < NQ:
                    fetch(q + 2)
                xq = xqs.pop(q)
                ob = outp.tile([128, WQ * 8 * CH], F32)
                for i in range(WQ):
                    xw = xq[:, i * 512:(i + 1) * 512]
                    hp = ps_h.tile([E, WIN], F32)
                    # K=128 with zero-padded weight halves -> plain mode
                    nc.tensor.matmul(hp[:, 0:512], wtd_t[:, 0:D], xw,
                                     start=True, stop=True)
                    nc.tensor.matmul(hp[:, 512:1024], wtd_t[:, D:2 * D], xw,
                                     start=True, stop=True)
                    hh = hhp.tile([128, WIN], F16)
                    nc.scalar.activation(hh[0:64, :], hp, AF.Tanh,
                                         bias=bvec_t[:, :], scale=1.0)
                    nc.vector.tensor_mul(hh[64:128, :], hh[0:64, :],
                                         hh[0:64, :])
                    po = ps_o.tile([128, WIN], F32)
                    for c in range(8):
                        nc.tensor.matmul(po[:, c * 128:c * 128 + CH],
                                         hh[:, c * 128:(c + 1) * 128],
                                         up_t[:, :], start=True, stop=True)
                    obw = ob[:, i * 8 * CH:(i + 1) * 8 * CH].rearrange(
                        "p (c ch) -> p c ch", c=8)
                    po_v = po.rearrange("p (c j) -> p c j", c=8)[:, :, 0:CH]
                    cb_v = cb_t.rearrange("p (c ch) -> p c ch", c=8)
                    nc.vector.tensor_add(obw, po_v, cb_v)
                nc.sync.dma_start(
                    out=out_r[q],
                    in_=ob.rearrange("p (i c ch) -> p i c ch", i=WQ, c=8),
                )
    nc.compile()
    return nc


def _hypernet(t, W1, b1, W2, b2, W3, b3):
    p = np.tanh(t.reshape(1, 1) @ W1 + b1)
    p = np.tanh(p @ W2 + b2)
    p = (p @ W3 + b3).reshape(-1).astype(np.float32)
    W = p[:BLOCK].reshape(E, D)
    U = p[BLOCK:2 * BLOCK].reshape(E, D)
    G = 1.0 / (1.0 + np.exp(-p[2 * BLOCK:3 * BLOCK].reshape(E, D)))
    U = (U * G).astype(np.float32)
    B = p[3 * BLOCK:].reshape(E, 1).astype(np.float32)
    return W.astype(np.float32), U, B


def _host_layout_x(x):
    """[N, D] -> per-core device layouts [NCORES][128, NSH//2].

    Sample index within a 1024-window: 8*p + 4*s + a (p<128, s<2, a<4);
    stored at partition (s*64+d), column (w*512 + a*128 + p).
    """
    xs = x.reshape(NCORES, NWIN, 128, 2, 4, D)        # [core, w, p, s, a, d]
    xs = xs.transpose(0, 3, 5, 1, 4, 2)               # [core, s, d, w, a, p]
    return np.ascontiguousarray(xs).reshape(NCORES, 128, NSH // 2)


def kernel(t, x, W1, b1, W2, b2, W3, b3):
    W, U, B = _hypernet(
        np.asarray(t, np.float32), np.asarray(W1, np.float32),
        np.asarray(b1, np.float32), np.asarray(W2, np.float32),
        np.asarray(b2, np.float32), np.asarray(W3, np.float32),
        np.asarray(b3, np.float32),
    )
    wu = np.sum(W * U, axis=1).astype(np.float32)      # [E]

    wtd = np.zeros((128, 2 * D), np.float32)
    wtd[0:64, 0:D] = W.T
    wtd[64:128, D:2 * D] = W.T
    up = np.zeros((128, CH), np.float32)
    up[0:E, 0:D] = U / E
    up[E:128, D] = wu / E
    up = up.astype(np.float16)
    cb = np.zeros((128, 8 * CH), np.float32)
    cb[:, D::CH] = -np.sum(wu) / E
    bvec = B.reshape(E, 1).astype(np.float32)

    xl = _host_layout_x(np.asarray(x, np.float32))

    if "nc" not in _CACHED:
        _CACHED["nc"] = _build_nc()
    nc = _CACHED["nc"]

    in_maps = [
        {"xt": xl[c], "wtd": wtd, "up": up, "bvec": bvec, "cb": cb}
        for c in range(NCORES)
    ]
    res = run_bass_kernel_spmd(nc, in_maps, core_ids=list(range(NCORES)))
    outs = [res.results[c]["out"] for c in range(NCORES)]
    return np.concatenate(outs, axis=0)


# revision 36
# speedup vs baseline: 1.0176x; 1.0176x over previous
"""Trainium2 Bass kernel for nn_CNF_76355928588411.

Data-parallel over N across 8 NeuronCores. The tiny t-conditioned hypernet
(three dense layers -> W, U, gate, B; ~6.6 MFLOP, depends only on the scalar
t) is evaluated once on the host in fp32 and its ~49KB output is replicated
to all cores per the sharding hint. The N-compute (h = tanh(x@W^T + B)
[E=64, N], dx = h^T@U/E, plus the Jacobian-trace column; ~4.3 GFLOP and
~130MB of I/O) runs on the devices.

Per-core device pipeline, per window of 1024 samples (2 subs x 512 cols):
  mm1 x2 (f32r, K=128 zero-padded weight halves) -> psum hp [64, 1024]
  ACT tanh(+B per-partition bias)                -> H[0:64]    (fp16)
  DVE square (cross-partition-offset write)      -> H[64:128] = h^2 (fp16)
  mm2 x8: lhsT = H[:, 128c:128c+128] (fp16, K=128, M=128 samples),
          rhs = up' [128, 65] fp16               -> psum [128sample, 65ch]
  DVE tensor-tensor add (+const tile carrying -mean(wu) in channel 64)
                                                 -> out sbuf [128, 520] f32
  DMA out (2080B contiguous per partition via 8-way sample interleave)

up' rows 0:64 = [U/E | 0], rows 64:128 = [0 | wu/E], so one K=128 matmul
emits dx and the h^2-weighted Jacobian column together. All matmuls are
plain 128x128 mode (no tile_position) so the PE never switches tiling
modes. x stays f32r (fp16 x halves DMA but measured 1.38e-3 rel err vs
9.2e-4; kept the safer dtype). Host pre-permutes x so device loads are
plain slices and output stores are contiguous runs; x-batch DMAs are
issued ahead of the constant loads and a dummy tanh hoists the ACT table
load to t=0. TimelineSim: ~60.3us/core; DMA-bound (48us busy).
"""

import sys

sys.path.insert(0, "/opt/trn_rl_repo")

import numpy as np

import concourse.bass as bass
from concourse import bacc
import concourse.mybir as mybir
import concourse.tile as tile
from concourse.bass_utils import run_bass_kernel_spmd

F32 = mybir.dt.float32
F32R = mybir.dt.float32r
F16 = mybir.dt.float16
AF = mybir.ActivationFunctionType

E, D, H_DIM, N = 64, 64, 512, 262144
BLOCK = E * D
OUT_DIM = 3 * BLOCK + E
NCORES = 8
NSH = N // NCORES          # 32768 samples per core
WIN = 1024                 # samples per window
NWIN = NSH // WIN          # 32 windows
WQ = 2                     # windows per DMA batch
NQ = NWIN // WQ            # 8 DMA batches
CH = D + 1                 # 65 output channels

_CACHED = {}


def _build_nc():
    nc = bacc.Bacc("TRN2", target_bir_lowering=False, debug=False,
                   num_devices=NCORES)
    xt = nc.dram_tensor("xt", [128, NSH // 2], F32R, kind="ExternalInput")
    wtd = nc.dram_tensor("wtd", [128, 2 * D], F32R, kind="ExternalInput")
    up = nc.dram_tensor("up", [128, CH], F16, kind="ExternalInput")
    bvec = nc.dram_tensor("bvec", [E, 1], F32, kind="ExternalInput")
    cb = nc.dram_tensor("cb", [128, 8 * CH], F32, kind="ExternalInput")
    out = nc.dram_tensor("out", [NSH, CH], F32, kind="ExternalOutput")

    # out row = 1024*(q*WQ+i) + 8*p + c
    out_r = out.ap().rearrange("(q i p c) ch -> q p i c ch", i=WQ, p=128, c=8)

    with tile.TileContext(nc) as tc:
        with (
            tc.tile_pool(name="consts", bufs=1) as consts,
            tc.tile_pool(name="xin", bufs=8) as xin,
            tc.tile_pool(name="hh", bufs=3) as hhp,
            tc.tile_pool(name="outp", bufs=6) as outp,
            tc.tile_pool(name="ps_h", bufs=2, space="PSUM") as ps_h,
            tc.tile_pool(name="ps_o", bufs=2, space="PSUM") as ps_o,
        ):
            wtd_t = consts.tile([128, 2 * D], F32R)  # cols 0:64=[WT;0], 64:128=[0;WT]
            up_t = consts.tile([128, CH], F16)
            bvec_t = consts.tile([E, 1], F32)
            cb_t = consts.tile([128, 8 * CH], F32)
            xqs = {}

            def fetch(q):
                xq_t = xin.tile([128, WQ * 512], F32R, tag="xq")
                xqs[q] = xq_t
                nc.sync.dma_start(
                    out=xq_t, in_=xt[:, q * WQ * 512:(q + 1) * WQ * 512]
                )

            fetch(0)
            fetch(1)
            fetch(2)
            fetch(3)
            dummy = consts.tile([1, 2], F32)
            nc.vector.memset(dummy, 0.0)
            nc.scalar.activation(dummy[:, 1:2], dummy[:, 0:1], AF.Tanh)
            nc.sync.dma_start(out=wtd_t, in_=wtd[:, :])
            nc.sync.dma_start(out=up_t, in_=up[:, :])
            nc.sync.dma_start(out=bvec_t, in_=bvec[:, :])
            nc.sync.dma_start(out=cb_t, in_=cb[:, :])

            for q in range(NQ):
                if q + 4 < NQ:
                    fetch(q + 4)
                xq = xqs.pop(q)
                ob = outp.tile([128, WQ * 8 * CH], F32)
                for i in range(WQ):
                    xw = xq[:, i * 512:(i + 1) * 512]
                    hp = ps_h.tile([E, WIN], F32)
                    # K=128 with zero-padded weight halves -> plain mode
                    nc.tensor.matmul(hp[:, 0:512], wtd_t[:, 0:D], xw,
                                     start=True, stop=True)
                    nc.tensor.matmul(hp[:, 512:1024], wtd_t[:, D:2 * D], xw,
                                     start=True, stop=True)
                    hh = hhp.tile([128, WIN], F16)
                    nc.scalar.activation(hh[0:64, :], hp, AF.Tanh,
                                         bias=bvec_t[:, :], scale=1.0)
                    nc.vector.tensor_mul(hh[64:128, :], hh[0:64, :],
                                         hh[0:64, :])
                    po = ps_o.tile([128, WIN], F32)
                    for c in range(8):
                        nc.tensor.matmul(po[:, c * 128:c * 128 + CH],
                                         hh[:, c * 128:(c + 1) * 128],
                                         up_t[:, :], start=True, stop=True)
                    obw = ob[:, i * 8 * CH:(i + 1) * 8 * CH].rearrange(
                        "p (c ch) -> p c ch", c=8)
                    po_v = po.rearrange("p (c j) -> p c j", c=8)[:, :, 0:CH]
                    cb_v = cb_t.rearrange("p (c ch) -> p c ch", c=8)
                    nc.vector.tensor_add(obw, po_v, cb_v)
                nc.sync.dma_start(
                    out=out_r[q],
                    in_=ob.rearrange("p (i c ch) -> p i c ch", i=WQ, c=8),
                )
    nc.compile()
    return nc


def _hypernet(t, W1, b1, W2, b2, W3, b3):
    p = np.tanh(t.reshape(1, 1) @ W1 + b1)
    p = np.tanh(p @ W2 + b2)
    p = (p @ W3 + b3).reshape(-1).astype(np.float32)
    W = p[:BLOCK].reshape(E, D)
    U = p[BLOCK:2 * BLOCK].reshape(E, D)
    G = 1.0 / (1.0 + np.exp(-p[2 * BLOCK:3 * BLOCK].reshape(E, D)))
    U = (U * G).astype(np.float32)
    B = p[3 * BLOCK:].reshape(E, 1).astype(np.float32)
    return W.astype(np.float32), U, B


def _host_layout_x(x):
    """[N, D] -> per-core device layouts [NCORES][128, NSH//2].

    Sample index within a 1024-window: 8*p + 4*s + a (p<128, s<2, a<4);
    stored at partition (s*64+d), column (w*512 + a*128 + p).
    """
    xs = x.reshape(NCORES, NWIN, 128, 2, 4, D)        # [core, w, p, s, a, d]
    xs = xs.transpose(0, 3, 5, 1, 4, 2)               # [core, s, d, w, a, p]
    return np.ascontiguousarray(xs).reshape(NCORES, 128, NSH // 2)


def kernel(t, x, W1, b1, W2, b2, W3, b3):
    W, U, B = _hypernet(
        np.asarray(t, np.float32), np.asarray(W1, np.float32),
        np.asarray(b1, np.float32), np.asarray(W2, np.float32),
        np.asarray(b2, np.float32), np.asarray(W3, np.float32),
        np.asarray(b3, np.float32),
    )
    wu = np.sum(W * U, axis=1).astype(np.float32)      # [E]

    wtd = np.zeros((128, 2 * D), np.float32)
    wtd[0:64, 0:D] = W.T
    wtd[64:128, D:2 * D] = W.T
    up = np.zeros((128, CH), np.float32)
    up[0:E, 0:D] = U / E
    up[E:128, D] = wu / E
    up = up.astype(np.float16)
    cb = np.zeros((128, 8 * CH), np.float32)
    cb[:, D::CH] = -np.sum(wu) / E
    bvec = B.reshape(E, 1).astype(np.float32)

    xl = _host_layout_x(np.asarray(x, np.float32))

    if "nc" not in _CACHED:
        _CACHED["nc"] = _build_nc()
    nc = _CACHED["nc"]

    in_maps = [
        {"xt": xl[c], "wtd": wtd, "up": up, "bvec": bvec, "cb": cb}
        for c in range(NCORES)
    ]
    res = run_bass_kernel_spmd(nc, in_maps, core_ids=list(range(NCORES)))
    outs = [res.results[c]["out"] for c in range(NCORES)]
    return np.concatenate(outs, axis=0)


# revision 47
# speedup vs baseline: 1.0996x; 1.0806x over previous
"""Trainium2 Bass kernel for nn_CNF_76355928588411.

Data-parallel over N across 8 NeuronCores. The tiny t-conditioned hypernet
(three dense layers -> W, U, gate, B; ~6.6 MFLOP, depends only on the scalar
t) is evaluated once on the host in fp32 and its ~49KB output is replicated
to all cores per the sharding hint. The N-compute (h = tanh(x@W^T + B)
[E=64, N], dx = h^T@U/E, plus the Jacobian-trace column; ~4.3 GFLOP and
~130MB of I/O) runs on the devices.

Per-core device pipeline, per window of 1024 samples (2 subs x 512 cols):
  mm1 x2 (f32r, K=128 zero-padded weight halves) -> psum hp [64, 1024]
  ACT tanh(+B per-partition bias)                -> H[0:64]    (fp16)
  DVE square (cross-partition-offset write)      -> H[64:128] = h^2 (fp16)
  mm2 x8: lhsT = H[:, 128c:128c+128] (fp16, K=128, M=128 samples),
          rhs = up' [128, 65] fp16               -> psum [128sample, 65ch]
  DVE tensor-tensor add (+const tile carrying -mean(wu) in channel 64)
                                                 -> out sbuf [128, 520] f32
  DMA out (2080B contiguous per partition via 8-way sample interleave)

up' rows 0:64 = [U/E | 0], rows 64:128 = [0 | wu/E], so one K=128 matmul
emits dx and the h^2-weighted Jacobian column together. All matmuls are
plain 128x128 mode (no tile_position) so the PE never switches tiling
modes. x stays f32r (fp16 x halves DMA but measured 1.38e-3 rel err vs
9.2e-4; kept the safer dtype). Host pre-permutes x so device loads are
plain slices and output stores are contiguous runs; x-batch DMAs are
issued ahead of the constant loads and a dummy tanh hoists the ACT table
load to t=0. TimelineSim: ~54.8us/core; DMA-bound (48us busy, ~93% utilization).
"""

import sys

sys.path.insert(0, "/opt/trn_rl_repo")

import numpy as np

import concourse.bass as bass
from concourse import bacc
import concourse.mybir as mybir
import concourse.tile as tile
from concourse.bass_utils import run_bass_kernel_spmd

F32 = mybir.dt.float32
F32R = mybir.dt.float32r
F16 = mybir.dt.float16
AF = mybir.ActivationFunctionType

E, D, H_DIM, N = 64, 64, 512, 262144
BLOCK = E * D
OUT_DIM = 3 * BLOCK + E
NCORES = 8
NSH = N // NCORES          # 32768 samples per core
WIN = 1024                 # samples per window
NWIN = NSH // WIN          # 32 windows
WQ = 2                     # windows per DMA batch
NQ = NWIN // WQ            # 8 DMA batches
CH = D + 1                 # 65 output channels

_CACHED = {}


def _build_nc():
    nc = bacc.Bacc("TRN2", target_bir_lowering=False, debug=False,
                   num_devices=NCORES)
    xt = nc.dram_tensor("xt", [128, NSH // 2], F32R, kind="ExternalInput")
    wtd = nc.dram_tensor("wtd", [128, 2 * D], F32R, kind="ExternalInput")
    up = nc.dram_tensor("up", [128, CH], F16, kind="ExternalInput")
    bvec = nc.dram_tensor("bvec", [E, 1], F32, kind="ExternalInput")
    cb = nc.dram_tensor("cb", [128, 8 * CH], F32, kind="ExternalInput")
    out = nc.dram_tensor("out", [NSH, CH], F32, kind="ExternalOutput")

    # out row = 1024*w + 8*p + c
    out_r = out.ap().rearrange("(w p c) ch -> w p c ch", p=128, c=8)

    with tile.TileContext(nc) as tc:
        with (
            tc.tile_pool(name="consts", bufs=1) as consts,
            tc.tile_pool(name="xin", bufs=8) as xin,
            tc.tile_pool(name="hh", bufs=4) as hhp,
            tc.tile_pool(name="outp", bufs=6) as outp,
            tc.tile_pool(name="ps_h", bufs=2, space="PSUM") as ps_h,
            tc.tile_pool(name="ps_o", bufs=2, space="PSUM") as ps_o,
        ):
            wtd_t = consts.tile([128, 2 * D], F32R)  # cols 0:64=[WT;0], 64:128=[0;WT]
            up_t = consts.tile([128, CH], F16)
            bvec_t = consts.tile([E, 1], F32)
            cb_t = consts.tile([128, 8 * CH], F32)
            xqs = {}

            def fetch(q, split=False):
                xq_t = xin.tile([128, WQ * 512], F32R, tag="xq")
                xqs[q] = xq_t
                lo = q * WQ * 512
                if split:
                    nc.sync.dma_start(out=xq_t[:, 0:512],
                                      in_=xt[:, lo:lo + 512])
                    nc.sync.dma_start(out=xq_t[:, 512:WQ * 512],
                                      in_=xt[:, lo + 512:lo + WQ * 512])
                else:
                    nc.sync.dma_start(out=xq_t, in_=xt[:, lo:lo + WQ * 512])

            fetch(0, split=True)
            dummy = consts.tile([1, 2], F32)
            nc.vector.memset(dummy, 0.0)
            nc.scalar.activation(dummy[:, 1:2], dummy[:, 0:1], AF.Tanh)
            nc.sync.dma_start(out=wtd_t, in_=wtd[:, :])
            nc.sync.dma_start(out=up_t, in_=up[:, :])
            nc.sync.dma_start(out=bvec_t, in_=bvec[:, :])
            nc.sync.dma_start(out=cb_t, in_=cb[:, :])
            fetch(1)
            fetch(2)
            fetch(3)

            for q in range(NQ):
                if q + 4 < NQ:
                    fetch(q + 4)
                xq = xqs.pop(q)
                for i in range(WQ):
                    ob = outp.tile([128, 8 * CH], F32)
                    xw = xq[:, i * 512:(i + 1) * 512]
                    hp = ps_h.tile([E, WIN], F32)
                    # K=128 with zero-padded weight halves -> plain mode
                    nc.tensor.matmul(hp[:, 0:512], wtd_t[:, 0:D], xw,
                                     start=True, stop=True)
                    nc.tensor.matmul(hp[:, 512:1024], wtd_t[:, D:2 * D], xw,
                                     start=True, stop=True)
                    hh = hhp.tile([128, WIN], F16)
                    nc.scalar.activation(hh[0:64, :], hp, AF.Tanh,
                                         bias=bvec_t[:, :], scale=1.0)
                    nc.vector.tensor_mul(hh[64:128, :], hh[0:64, :],
                                         hh[0:64, :])
                    po = ps_o.tile([128, WIN], F32)
                    for c in range(8):
                        nc.tensor.matmul(po[:, c * 128:c * 128 + CH],
                                         hh[:, c * 128:(c + 1) * 128],
                                         up_t[:, :], start=True, stop=True)
                    obw = ob.rearrange("p (c ch) -> p c ch", c=8)
                    po_v = po.rearrange("p (c j) -> p c j", c=8)[:, :, 0:CH]
                    cb_v = cb_t.rearrange("p (c ch) -> p c ch", c=8)
                    nc.vector.tensor_add(obw, po_v, cb_v)
                    nc.sync.dma_start(
                        out=out_r[q * WQ + i],
                        in_=ob.rearrange("p (c ch) -> p c ch", c=8),
                    )
    nc.compile()
    return nc


def _hypernet(t, W1, b1, W2, b2, W3, b3):
    p = np.tanh(t.reshape(1, 1) @ W1 + b1)
    p = np.tanh(p @ W2 + b2)
    p = (p @ W3 + b3).reshape(-1).astype(np.float32)
    W = p[:BLOCK].reshape(E, D)
    U = p[BLOCK:2 * BLOCK].reshape(E, D)
    G = 1.0 / (1.0 + np.exp(-p[2 * BLOCK:3 * BLOCK].reshape(E, D)))
    U = (U * G).astype(np.float32)
    B = p[3 * BLOCK:].reshape(E, 1).astype(np.float32)
    return W.astype(np.float32), U, B


def _host_layout_x(x):
    """[N, D] -> per-core device layouts [NCORES][128, NSH//2].

    Sample index within a 1024-window: 8*p + 4*s + a (p<128, s<2, a<4);
    stored at partition (s*64+d), column (w*512 + a*128 + p).
    """
    xs = x.reshape(NCORES, NWIN, 128, 2, 4, D)        # [core, w, p, s, a, d]
    xs = xs.transpose(0, 3, 5, 1, 4, 2)               # [core, s, d, w, a, p]
    return np.ascontiguousarray(xs).reshape(NCORES, 128, NSH // 2)


def kernel(t, x, W1, b1, W2, b2, W3, b3):
    W, U, B = _hypernet(
        np.asarray(t, np.float32), np.asarray(W1, np.float32),
        np.asarray(b1, np.float32), np.asarray(W2, np.float32),
        np.asarray(b2, np.float32), np.asarray(W3, np.float32),
        np.asarray(b3, np.float32),
    )
    wu = np.sum(W * U, axis=1).astype(np.float32)      # [E]

    wtd = np.zeros((128, 2 * D), np.float32)
    wtd[0:64, 0:D] = W.T
    wtd[64:128, D:2 * D] = W.T
    up = np.zeros((128, CH), np.float32)
    up[0:E, 0:D] = U / E
    up[E:128, D] = wu / E
    up = up.astype(np.float16)
    cb = np.zeros((128, 8 * CH), np.float32)
    cb[:, D::CH] = -np.sum(wu) / E
    bvec = B.reshape(E, 1).astype(np.float32)

    xl = _host_layout_x(np.asarray(x, np.float32))

    if "nc" not in _CACHED:
        _CACHED["nc"] = _build_nc()
    nc = _CACHED["nc"]

    in_maps = [
        {"xt": xl[c], "wtd": wtd, "up": up, "bvec": bvec, "cb": cb}
        for c in range(NCORES)
    ]
    res = run_bass_kernel_spmd(nc, in_maps, core_ids=list(range(NCORES)))
    outs = [res.results[c]["out"] for c in range(NCORES)]
    return np.concatenate(outs, axis=0)


# revision 55
# speedup vs baseline: 1.1188x; 1.0175x over previous
"""Trainium2 Bass kernel for nn_CNF_76355928588411.

Data-parallel over N across 8 NeuronCores. The tiny t-conditioned hypernet
(three dense layers -> W, U, gate, B; ~6.6 MFLOP, depends only on the scalar
t) is evaluated once on the host in fp32 and its ~49KB output is replicated
to all cores per the sharding hint. The N-compute (h = tanh(x@W^T + B)
[E=64, N], dx = h^T@U/E, plus the Jacobian-trace column; ~4.3 GFLOP and
~130MB of I/O) runs on the devices.

Per-core device pipeline, per window of 1024 samples (2 subs x 512 cols):
  mm1 x2 (f32r, K=128 zero-padded weight halves) -> psum hp [64, 1024]
  ACT tanh(+B per-partition bias)                -> H[0:64]    (fp16)
  DVE square (cross-partition-offset write)      -> H[64:128] = h^2 (fp16)
  mm2 x8: lhsT = H[:, 128c:128c+128] (fp16, K=128, M=128 samples),
          rhs = up' [128, 65] fp16               -> psum [128sample, 65ch]
  DVE tensor-tensor add (+const tile carrying -mean(wu) in channel 64)
                                                 -> out sbuf [128, 520] f32
  DMA out (2080B contiguous per partition via 8-way sample interleave)

up' rows 0:64 = [U/E | 0], rows 64:128 = [0 | wu/E], so one K=128 matmul
emits dx and the h^2-weighted Jacobian column together. All matmuls are
plain 128x128 mode (no tile_position) so the PE never switches tiling
modes. x stays f32r (fp16 x halves DMA but measured 1.38e-3 rel err vs
9.2e-4; kept the safer dtype). Host pre-permutes x so device loads are
plain slices and output stores are contiguous runs; x-batch DMAs are
issued ahead of the constant loads and a dummy tanh hoists the ACT table
load to t=0. TimelineSim: ~53.9us/core; DMA-bound (48us busy, ~95% stream utilization).
"""

import sys

sys.path.insert(0, "/opt/trn_rl_repo")

import numpy as np

import concourse.bass as bass
from concourse import bacc
import concourse.mybir as mybir
import concourse.tile as tile
from concourse.bass_utils import run_bass_kernel_spmd

F32 = mybir.dt.float32
F32R = mybir.dt.float32r
F16 = mybir.dt.float16
AF = mybir.ActivationFunctionType

E, D, H_DIM, N = 64, 64, 512, 262144
BLOCK = E * D
OUT_DIM = 3 * BLOCK + E
NCORES = 8
NSH = N // NCORES          # 32768 samples per core
WIN = 1024                 # samples per window
NWIN = NSH // WIN          # 32 windows
WQ = 2                     # windows per DMA batch
NQ = NWIN // WQ            # 8 DMA batches
CH = D + 1                 # 65 output channels

_CACHED = {}


def _build_nc():
    nc = bacc.Bacc("TRN2", target_bir_lowering=False, debug=False,
                   num_devices=NCORES)
    xt = nc.dram_tensor("xt", [128, NSH // 2], F32R, kind="ExternalInput")
    wtd = nc.dram_tensor("wtd", [128, 2 * D], F32R, kind="ExternalInput")
    up = nc.dram_tensor("up", [128, CH], F16, kind="ExternalInput")
    bvec = nc.dram_tensor("bvec", [E, 1], F32, kind="ExternalInput")
    cb = nc.dram_tensor("cb", [128, 8 * CH], F32, kind="ExternalInput")
    out = nc.dram_tensor("out", [NSH, CH], F32, kind="ExternalOutput")

    # out row = 1024*w + 8*p + c
    out_r = out.ap().rearrange("(w p c) ch -> w p c ch", p=128, c=8)

    with tile.TileContext(nc) as tc:
        with (
            tc.tile_pool(name="consts", bufs=1) as consts,
            tc.tile_pool(name="xin", bufs=8) as xin,
            tc.tile_pool(name="hh", bufs=4) as hhp,
            tc.tile_pool(name="outp", bufs=6) as outp,
            tc.tile_pool(name="ps_h", bufs=2, space="PSUM") as ps_h,
            tc.tile_pool(name="ps_o", bufs=2, space="PSUM") as ps_o,
        ):
            wtd_t = consts.tile([128, 2 * D], F32R)  # cols 0:64=[WT;0], 64:128=[0;WT]
            up_t = consts.tile([128, CH], F16)
            bvec_t = consts.tile([E, 1], F32)
            cb_t = consts.tile([128, 8 * CH], F32)
            xqs = {}

            def fetch(q, split=False):
                xq_t = xin.tile([128, WQ * 512], F32R, tag="xq")
                xqs[q] = xq_t
                lo = q * WQ * 512
                if split:
                    nc.sync.dma_start(out=xq_t[:, 0:512],
                                      in_=xt[:, lo:lo + 512])
                    nc.sync.dma_start(out=xq_t[:, 512:WQ * 512],
                                      in_=xt[:, lo + 512:lo + WQ * 512])
                else:
                    nc.sync.dma_start(out=xq_t, in_=xt[:, lo:lo + WQ * 512])

            fetch(0, split=True)
            dummy = consts.tile([1, 2], F32)
            nc.vector.memset(dummy, 0.0)
            nc.scalar.activation(dummy[:, 1:2], dummy[:, 0:1], AF.Tanh)
            nc.sync.dma_start(out=wtd_t, in_=wtd[:, :])
            nc.sync.dma_start(out=up_t, in_=up[:, :])
            nc.sync.dma_start(out=bvec_t, in_=bvec[:, :])
            nc.sync.dma_start(out=cb_t, in_=cb[:, :])
            fetch(1)
            fetch(2)
            fetch(3)
            fetch(4)

            for q in range(NQ):
                if q + 5 < NQ:
                    fetch(q + 5)
                xq = xqs.pop(q)
                for i in range(WQ):
                    ob = outp.tile([128, 8 * CH], F32)
                    xw = xq[:, i * 512:(i + 1) * 512]
                    hp = ps_h.tile([E, WIN], F32)
                    # K=128 with zero-padded weight halves -> plain mode
                    nc.tensor.matmul(hp[:, 0:512], wtd_t[:, 0:D], xw,
                                     start=True, stop=True)
                    nc.tensor.matmul(hp[:, 512:1024], wtd_t[:, D:2 * D], xw,
                                     start=True, stop=True)
                    hh = hhp.tile([128, WIN], F16)
                    nc.scalar.activation(hh[0:64, :], hp, AF.Tanh,
                                         bias=bvec_t[:, :], scale=1.0)
                    nc.vector.tensor_mul(hh[64:128, :], hh[0:64, :],
                                         hh[0:64, :])
                    po = ps_o.tile([128, WIN], F32)
                    for c in range(8):
                        nc.tensor.matmul(po[:, c * 128:c * 128 + CH],
                                         hh[:, c * 128:(c + 1) * 128],
                                         up_t[:, :], start=True, stop=True)
                    obw = ob.rearrange("p (c ch) -> p c ch", c=8)
                    po_v = po.rearrange("p (c j) -> p c j", c=8)[:, :, 0:CH]
                    cb_v = cb_t.rearrange("p (c ch) -> p c ch", c=8)
                    nc.vector.tensor_add(obw, po_v, cb_v)
                    nc.sync.dma_start(
                        out=out_r[q * WQ + i],
                        in_=ob.rearrange("p (c ch) -> p c ch", c=8),
                    )
    nc.compile()
    return nc


def _hypernet(t, W1, b1, W2, b2, W3, b3):
    p = np.tanh(t.reshape(1, 1) @ W1 + b1)
    p = np.tanh(p @ W2 + b2)
    p = (p @ W3 + b3).reshape(-1).astype(np.float32)
    W = p[:BLOCK].reshape(E, D)
    U = p[BLOCK:2 * BLOCK].reshape(E, D)
    G = 1.0 / (1.0 + np.exp(-p[2 * BLOCK:3 * BLOCK].reshape(E, D)))
    U = (U * G).astype(np.float32)
    B = p[3 * BLOCK:].reshape(E, 1).astype(np.float32)
    return W.astype(np.float32), U, B


def _host_layout_x(x):
    """[N, D] -> per-core device layouts [NCORES][128, NSH//2].

    Sample index within a 1024-window: 8*p + 4*s + a (p<128, s<2, a<4);
    stored at partition (s*64+d), column (w*512 + a*128 + p).
    """
    xs = x.reshape(NCORES, NWIN, 128, 2, 4, D)        # [core, w, p, s, a, d]
    xs = xs.transpose(0, 3, 5, 1, 4, 2)               # [core, s, d, w, a, p]
    return np.ascontiguousarray(xs).reshape(NCORES, 128, NSH // 2)


def kernel(t, x, W1, b1, W2, b2, W3, b3):
    W, U, B = _hypernet(
        np.asarray(t, np.float32), np.asarray(W1, np.float32),
        np.asarray(b1, np.float32), np.asarray(W2, np.float32),
        np.asarray(b2, np.float32), np.asarray(W3, np.float32),
        np.asarray(b3, np.float32),
    )
    wu = np.sum(W * U, axis=1).astype(np.float32)      # [E]

    wtd = np.zeros((128, 2 * D), np.float32)
    wtd[0:64, 0:D] = W.T
    wtd[64:128, D:2 * D] = W.T
    up = np.zeros((128, CH), np.float32)
    up[0:E, 0:D] = U / E
    up[E:128, D] = wu / E
    up = up.astype(np.float16)
    cb = np.zeros((128, 8 * CH), np.float32)
    cb[:, D::CH] = -np.sum(wu) / E
    bvec = B.reshape(E, 1).astype(np.float32)

    xl = _host_layout_x(np.asarray(x, np.float32))

    if "nc" not in _CACHED:
        _CACHED["nc"] = _build_nc()
    nc = _CACHED["nc"]

    in_maps = [
        {"xt": xl[c], "wtd": wtd, "up": up, "bvec": bvec, "cb": cb}
        for c in range(NCORES)
    ]
    res = run_bass_kernel_spmd(nc, in_maps, core_ids=list(range(NCORES)))
    outs = [res.results[c]["out"] for c in range(NCORES)]
    return np.concatenate(outs, axis=0)


# revision 60
# speedup vs baseline: 1.1314x; 1.0113x over previous
"""Trainium2 Bass kernel for nn_CNF_76355928588411.

Data-parallel over N across 8 NeuronCores. The tiny t-conditioned hypernet
(three dense layers -> W, U, gate, B; ~6.6 MFLOP, depends only on the scalar
t) is evaluated once on the host in fp32 and its ~49KB output is replicated
to all cores per the sharding hint. The N-compute (h = tanh(x@W^T + B)
[E=64, N], dx = h^T@U/E, plus the Jacobian-trace column; ~4.3 GFLOP and
~130MB of I/O) runs on the devices.

Per-core device pipeline, per window of 1024 samples (2 subs x 512 cols):
  mm1 x2 (f32r, K=128 zero-padded weight halves) -> psum hp [64, 1024]
  ACT tanh(+B per-partition bias)                -> H[0:64]    (fp16)
  DVE square (cross-partition-offset write)      -> H[64:128] = h^2 (fp16)
  mm2 x8: lhsT = H[:, 128c:128c+128] (fp16, K=128, M=128 samples),
          rhs = up' [128, 65] fp16               -> psum [128sample, 65ch]
  DVE tensor-tensor add (+const tile carrying -mean(wu) in channel 64)
                                                 -> out sbuf [128, 520] f32
  DMA out (2080B contiguous per partition via 8-way sample interleave)

up' rows 0:64 = [U/E | 0], rows 64:128 = [0 | wu/E], so one K=128 matmul
emits dx and the h^2-weighted Jacobian column together. All matmuls are
plain 128x128 mode (no tile_position) so the PE never switches tiling
modes. x stays f32r (fp16 x halves DMA but measured 1.38e-3 rel err vs
9.2e-4; kept the safer dtype). Host pre-permutes x so device loads are
plain slices and output stores are contiguous runs; x-batch DMAs are
issued ahead of the constant loads and a dummy tanh hoists the ACT table
load to t=0. TimelineSim: ~53.3us/core; DMA-bound (48us busy, ~96% stream utilization).
"""

import sys

sys.path.insert(0, "/opt/trn_rl_repo")

import numpy as np

import concourse.bass as bass
from concourse import bacc
import concourse.mybir as mybir
import concourse.tile as tile
from concourse.bass_utils import run_bass_kernel_spmd

F32 = mybir.dt.float32
F32R = mybir.dt.float32r
F16 = mybir.dt.float16
AF = mybir.ActivationFunctionType

E, D, H_DIM, N = 64, 64, 512, 262144
BLOCK = E * D
OUT_DIM = 3 * BLOCK + E
NCORES = 8
NSH = N // NCORES          # 32768 samples per core
WIN = 1024                 # samples per window
NWIN = NSH // WIN          # 32 windows
WQ = 2                     # windows per DMA batch
NQ = NWIN // WQ            # 8 DMA batches
CH = D + 1                 # 65 output channels

_CACHED = {}


def _build_nc():
    nc = bacc.Bacc("TRN2", target_bir_lowering=False, debug=False,
                   num_devices=NCORES)
    xt = nc.dram_tensor("xt", [128, NSH // 2], F32R, kind="ExternalInput")
    wtd = nc.dram_tensor("wtd", [128, 2 * D], F32R, kind="ExternalInput")
    up = nc.dram_tensor("up", [128, CH], F16, kind="ExternalInput")
    bvec = nc.dram_tensor("bvec", [E, 1], F32, kind="ExternalInput")
    cb = nc.dram_tensor("cb", [128, 8 * CH], F32, kind="ExternalInput")
    out = nc.dram_tensor("out", [NSH, CH], F32, kind="ExternalOutput")

    # out row = 1024*w + 8*p + c
    out_r = out.ap().rearrange("(w p c) ch -> w p c ch", p=128, c=8)

    with tile.TileContext(nc) as tc:
        with (
            tc.tile_pool(name="consts", bufs=1) as consts,
            tc.tile_pool(name="xin", bufs=8) as xin,
            tc.tile_pool(name="hh", bufs=4) as hhp,
            tc.tile_pool(name="outp", bufs=6) as outp,
            tc.tile_pool(name="ps_h", bufs=2, space="PSUM") as ps_h,
            tc.tile_pool(name="ps_o", bufs=2, space="PSUM") as ps_o,
        ):
            wtd_t = consts.tile([128, 2 * D], F32R)  # cols 0:64=[WT;0], 64:128=[0;WT]
            up_t = consts.tile([128, CH], F16)
            bvec_t = consts.tile([E, 1], F32)
            cb_t = consts.tile([128, 8 * CH], F32)
            xqs = {}

            def fetch(q, split=False):
                xq_t = xin.tile([128, WQ * 512], F32R, tag="xq")
                xqs[q] = xq_t
                lo = q * WQ * 512
                if split:
                    nc.sync.dma_start(out=xq_t[:, 0:512],
                                      in_=xt[:, lo:lo + 512])
                    nc.sync.dma_start(out=xq_t[:, 512:WQ * 512],
                                      in_=xt[:, lo + 512:lo + WQ * 512])
                else:
                    nc.sync.dma_start(out=xq_t, in_=xt[:, lo:lo + WQ * 512])

            fetch(0)
            dummy = consts.tile([1, 2], F32)
            nc.vector.memset(dummy, 0.0)
            nc.scalar.activation(dummy[:, 1:2], dummy[:, 0:1], AF.Tanh)
            nc.sync.dma_start(out=wtd_t, in_=wtd[:, :])
            nc.sync.dma_start(out=up_t, in_=up[:, :])
            nc.sync.dma_start(out=bvec_t, in_=bvec[:, :])
            nc.sync.dma_start(out=cb_t, in_=cb[:, :])
            fetch(1)
            fetch(2)
            fetch(3)
            fetch(4)

            for q in range(NQ):
                if q + 5 < NQ:
                    fetch(q + 5)
                xq = xqs.pop(q)
                for i in range(WQ):
                    ob = outp.tile([128, 8 * CH], F32)
                    xw = xq[:, i * 512:(i + 1) * 512]
                    hp = ps_h.tile([E, WIN], F32)
                    # K=128 with zero-padded weight halves -> plain mode
                    nc.tensor.matmul(hp[:, 0:512], wtd_t[:, 0:D], xw,
                                     start=True, stop=True)
                    nc.tensor.matmul(hp[:, 512:1024], wtd_t[:, D:2 * D], xw,
                                     start=True, stop=True)
                    hh = hhp.tile([128, WIN], F16)
                    nc.scalar.activation(hh[0:64, :], hp, AF.Tanh,
                                         bias=bvec_t[:, :], scale=1.0)
                    nc.vector.tensor_mul(hh[64:128, :], hh[0:64, :],
                                         hh[0:64, :])
                    po = ps_o.tile([128, WIN], F32)
                    for c in range(8):
                        nc.tensor.matmul(po[:, c * 128:c * 128 + CH],
                                         hh[:, c * 128:(c + 1) * 128],
                                         up_t[:, :], start=True, stop=True)
                    obw = ob.rearrange("p (c ch) -> p c ch", c=8)
                    po_v = po.rearrange("p (c j) -> p c j", c=8)[:, :, 0:CH]
                    cb_v = cb_t.rearrange("p (c ch) -> p c ch", c=8)
                    nc.vector.tensor_add(obw, po_v, cb_v)
                    nc.sync.dma_start(
                        out=out_r[q * WQ + i],
                        in_=ob.rearrange("p (c ch) -> p c ch", c=8),
                    )
    nc.compile()
    return nc


def _hypernet(t, W1, b1, W2, b2, W3, b3):
    p = np.tanh(t.reshape(1, 1) @ W1 + b1)
    p = np.tanh(p @ W2 + b2)
    p = (p @ W3 + b3).reshape(-1).astype(np.float32)
    W = p[:BLOCK].reshape(E, D)
    U = p[BLOCK:2 * BLOCK].reshape(E, D)
    G = 1.0 / (1.0 + np.exp(-p[2 * BLOCK:3 * BLOCK].reshape(E, D)))
    U = (U * G).astype(np.float32)
    B = p[3 * BLOCK:].reshape(E, 1).astype(np.float32)
    return W.astype(np.float32), U, B


def _host_layout_x(x):
    """[N, D] -> per-core device layouts [NCORES][128, NSH//2].

    Sample index within a 1024-window: 8*p + 4*s + a (p<128, s<2, a<4);
    stored at partition (s*64+d), column (w*512 + a*128 + p).
    """
    xs = x.reshape(NCORES, NWIN, 128, 2, 4, D)        # [core, w, p, s, a, d]
    xs = xs.transpose(0, 3, 5, 1, 4, 2)               # [core, s, d, w, a, p]
    return np.ascontiguousarray(xs).reshape(NCORES, 128, NSH // 2)


def kernel(t, x, W1, b1, W2, b2, W3, b3):
    W, U, B = _hypernet(
        np.asarray(t, np.float32), np.asarray(W1, np.float32),
        np.asarray(b1, np.float32), np.asarray(W2, np.float32),
        np.asarray(b2, np.float32), np.asarray(W3, np.float32),
        np.asarray(b3, np.float32),
    )
    wu = np.sum(W * U, axis=1).astype(np.float32)      # [E]

    wtd = np.zeros((128, 2 * D), np.float32)
    wtd[0:64, 0:D] = W.T
    wtd[64:128, D:2 * D] = W.T
    up = np.zeros((128, CH), np.float32)
    up[0:E, 0:D] = U / E
    up[E:128, D] = wu / E
    up = up.astype(np.float16)
    cb = np.zeros((128, 8 * CH), np.float32)
    cb[:, D::CH] = -np.sum(wu) / E
    bvec = B.reshape(E, 1).astype(np.float32)

    xl = _host_layout_x(np.asarray(x, np.float32))

    if "nc" not in _CACHED:
        _CACHED["nc"] = _build_nc()
    nc = _CACHED["nc"]

    in_maps = [
        {"xt": xl[c], "wtd": wtd, "up": up, "bvec": bvec, "cb": cb}
        for c in range(NCORES)
    ]
    res = run_bass_kernel_spmd(nc, in_maps, core_ids=list(range(NCORES)))
    outs = [res.results[c]["out"] for c in range(NCORES)]
    return np.concatenate(outs, axis=0)
